# revision 1
# baseline (speedup 1.0000x reference)
"""Trainium2 Bass kernel for DeformableConv2d (B,H,W,C=8,64,64,128; F=128; 3x3).

Strategy (data-parallel over batch, one batch element per NeuronCore):
  - Host: reshape inputs, precompute the (data-independent) base-grid constant
    and a channel-major weight layout. No data-dependent work on host.
  - Device, per core:
      * build x_pair in scratch DRAM: row q -> [x[q], x[q+64]] (zero padded),
        so one 512-element contiguous read at offset q*256 fetches the whole
        2x2 bilinear patch for integer corner q = y0*64 + x0.
      * index math on DVE: coords = grid + offset, clip, frac via mod,
        q = y0*64 + x0 (int32), 4 bilinear corner weights.
      * per pixel-tile (128 px): one indirect DMA gathers all 9 kernel
        points' patches [128, 9, 512]; DVE combines the 4 corners with
        per-partition scalar weights; PE transposes deform tiles into PSUM
        (channel-major); PE matmuls accumulate over the 9 kernel points into
        out^T; PE transposes back and results stream to DRAM.
"""

import os
from contextlib import ExitStack

import numpy as np

import concourse.bass as bass
import concourse.mybir as mybir
import concourse.tile as tile
from concourse import bacc
from concourse._compat import with_exitstack
from concourse.bass_utils import run_bass_kernel_spmd
from concourse.masks import make_identity

KH, KW, KN = 3, 3, 9
H = W_IMG = 64
C = 128
F = 128
P = 128
NPIX = H * W_IMG            # 4096 pixels per core
NT = NPIX // P              # 32 pixel tiles
NG = NT // 4                # 8 groups of 512 pixels
XROWS = NPIX + 1            # x rows incl. one zero pad row (host-padded)

f32 = mybir.dt.float32
bf16 = mybir.dt.bfloat16
i32 = mybir.dt.int32
ALU = mybir.AluOpType
ACT = mybir.ActivationFunctionType


def _grid_const():
    """reference._grid_offset in numpy, flattened to [4096, 18] then wrapped
    to the [128 partitions, 32*18] on-chip layout."""
    init = np.stack(np.meshgrid(np.arange(KH), np.arange(KW), indexing="ij"))
    init = init.reshape(-1, 2).astype(np.float32)
    ph, pw = (KH - 1) // 2, (KW - 1) // 2
    g = np.stack(
        np.meshgrid(np.arange(-ph, H - ph), np.arange(-pw, W_IMG - pw), indexing="ij"),
        axis=-1,
    ).astype(np.float32)
    full = (g[:, :, None, :] + init[None, None]).reshape(NPIX, 2 * KN)
    return np.ascontiguousarray(
        full.reshape(NT, P, 2 * KN).transpose(1, 0, 2).reshape(P, NT * 2 * KN)
    )


@with_exitstack
def _body(ctx: ExitStack, tc: "tile.TileContext", t_off, t_grid, t_w, t_b,
          t_out, t_xp, debug=False):
    nc = tc.nc
    off_ap = t_off.ap()
    grid_ap = t_grid.ap()
    w_ap = t_w.ap()
    b_ap = t_b.ap()
    out_ap = t_out.ap()
    xp_ap = t_xp.ap()

    const = ctx.enter_context(tc.tile_pool(name="const", bufs=1))
    idxp = ctx.enter_context(tc.tile_pool(name="idx", bufs=1))
    gpool = ctx.enter_context(tc.tile_pool(name="gath", bufs=8))
    dpool = ctx.enter_context(tc.tile_pool(name="deform", bufs=4))
    dTpool = ctx.enter_context(tc.tile_pool(name="dT", bufs=3))
    oTpool = ctx.enter_context(tc.tile_pool(name="oT", bufs=2))
    opool = ctx.enter_context(tc.tile_pool(name="o", bufs=4))
    ps_out = ctx.enter_context(tc.tile_pool(name="ps_out", bufs=2, space="PSUM"))
    ps_dT = ctx.enter_context(tc.tile_pool(name="ps_dT", bufs=2, space="PSUM"))
    ps_o = ctx.enter_context(tc.tile_pool(name="ps_o", bufs=2, space="PSUM"))

    # ---- constants ----
    ident = const.tile([P, P], f32)
    make_identity(nc, ident[:])
    ident16 = const.tile([P, P], bf16)
    nc.vector.tensor_copy(ident16[:], ident[:])
    w_sb = const.tile([P, KN, F], bf16)
    nc.sync.dma_start(w_sb[:], w_ap)  # [C, KN, F] bf16, c on partitions
    b_sb = const.tile([P, 1], f32)
    nc.sync.dma_start(b_sb[:], b_ap[:, None])

    xp3 = xp_ap.rearrange("q (j c) -> q j c", j=2)

    # ---- load offsets + grid ----
    offs = idxp.tile([P, NT, 2 * KN], f32)
    nc.sync.dma_start(offs[:], off_ap.rearrange("(t p) k -> p t k", p=P))
    grid = idxp.tile([P, NT, 2 * KN], f32)
    nc.sync.dma_start(grid[:], grid_ap.rearrange("p (t k) -> p t k", k=2 * KN))

    # ---- index math (all tiles at once) ----
    co = idxp.tile([P, NT, 2 * KN], f32)
    nc.vector.tensor_add(co[:], offs[:], grid[:])
    nc.vector.tensor_scalar(co[:], co[:], 0.0, float(H - 1), ALU.max, ALU.min)
    # floor via int round-trip; works for round-to-nearest (HW) and trunc (sim):
    # r = float(int(y)); floor = r - (r > y)
    ci = idxp.tile([P, NT, 2 * KN], i32)
    nc.vector.tensor_copy(ci[:], co[:])
    cf = idxp.tile([P, NT, 2 * KN], f32)
    nc.vector.tensor_copy(cf[:], ci[:])
    gt = idxp.tile([P, NT, 2 * KN], f32)
    nc.vector.tensor_tensor(gt[:], cf[:], co[:], ALU.is_gt)
    c0 = idxp.tile([P, NT, 2 * KN], f32)
    nc.vector.tensor_sub(c0[:], cf[:], gt[:])
    fr = idxp.tile([P, NT, 2 * KN], f32)
    nc.vector.tensor_sub(fr[:], co[:], c0[:])
    un = idxp.tile([P, NT, 2 * KN], f32)
    nc.vector.tensor_scalar(un[:], fr[:], -1.0, 1.0, ALU.mult, ALU.add)

    c0v = c0[:].rearrange("p t (n two) -> p t n two", two=2)
    frv = fr[:].rearrange("p t (n two) -> p t n two", two=2)
    unv = un[:].rearrange("p t (n two) -> p t n two", two=2)

    qf = idxp.tile([P, NT, KN], f32)
    nc.vector.scalar_tensor_tensor(
        qf[:], c0v[:, :, :, 0], 64.0, c0v[:, :, :, 1], ALU.mult, ALU.add
    )
    # kn-major int index tile; [128, 1] slices are contiguous for the DMA
    qi = idxp.tile([P, KN, NT], i32)
    nc.vector.tensor_copy(qi[:].rearrange("p n t -> p t n"), qf[:])

    # corner weights [00, 10, 01, 11]; rows ~ y (index 0), cols ~ x (index 1)
    w4 = idxp.tile([P, NT, KN, 4], f32)
    nc.vector.tensor_tensor(w4[:, :, :, 0], unv[:, :, :, 0], unv[:, :, :, 1], ALU.mult)
    nc.vector.tensor_tensor(w4[:, :, :, 1], frv[:, :, :, 0], unv[:, :, :, 1], ALU.mult)
    nc.vector.tensor_tensor(w4[:, :, :, 2], unv[:, :, :, 0], frv[:, :, :, 1], ALU.mult)
    nc.vector.tensor_tensor(w4[:, :, :, 3], frv[:, :, :, 0], frv[:, :, :, 1], ALU.mult)

    if debug:
        d_q = nc.dram_tensor("dbg_q", [P, KN * NG * 8], i32, kind="ExternalOutput")
        d_w4 = nc.dram_tensor("dbg_w4", [P, NT * KN * 4], f32, kind="ExternalOutput")
        d_g = nc.dram_tensor("dbg_g", [P, 4 * C], f32, kind="ExternalOutput")
        d_dt = nc.dram_tensor("dbg_dt", [P, 512], f32, kind="ExternalOutput")
        d_ot = nc.dram_tensor("dbg_ot", [P, 512], f32, kind="ExternalOutput")
        nc.sync.dma_start(d_q.ap().rearrange("p (n g j) -> p n g j", n=KN, g=NG), qall[:])
        nc.sync.dma_start(
            d_w4.ap().rearrange("p (t n j) -> p t n j", t=NT, n=KN), w4[:]
        )

    # ---- main loop ----
    for g in range(NG):
        ops = ps_out.tile([P, 512], f32)  # out^T accumulator [f, 512 px]
        for kn in range(KN):
            dps = ps_dT.tile([P, 512], bf16)  # deform^T [c, 512 px]
            for t4 in range(4):
                t = g * 4 + t4
                # one gather per (tile, kn): pair rows q, q+1 of x_pair =
                # corners [00 | 10 | 01 | 11], 2KB per descriptor
                G = gpool.tile([P, 4 * C], bf16)
                nc.gpsimd.indirect_dma_start(
                    out=G[:], out_offset=None, in_=xp3[:, :, :],
                    in_offset=bass.IndirectOffsetOnAxis(
                        ap=qi[:, kn, t : t + 1], axis=0),
                )
                d = dpool.tile([P, C], bf16)
                nc.vector.tensor_scalar_mul(d[:], G[:, 0:C], w4[:, t, kn, 0:1])
                for blk in (1, 2, 3):
                    nc.vector.scalar_tensor_tensor(
                        d[:],
                        G[:, blk * C : (blk + 1) * C],
                        w4[:, t, kn, blk : blk + 1],
                        d[:],
                        ALU.mult,
                        ALU.add,
                    )
                nc.tensor.transpose(dps[:, t4 * P : (t4 + 1) * P], d[:], ident16[:])
            dT = dTpool.tile([P, 512], bf16)
            nc.scalar.copy(dT[:], dps[:])
            nc.tensor.matmul(
                ops[:], lhsT=w_sb[:, kn, :], rhs=dT[:],
                start=(kn == 0), stop=(kn == KN - 1),
            )
        oT = oTpool.tile([P, 512], f32)
        nc.scalar.activation(oT[:], ops[:], ACT.Identity, bias=b_sb[:, 0:1], scale=1.0)
        if debug and g == 0:
            nc.sync.dma_start(d_ot.ap(), oT[:])
        for t4 in range(4):
            o_ps = ps_o.tile([P, P], f32)
            nc.tensor.transpose(o_ps[:], oT[:, t4 * P : (t4 + 1) * P], ident[:])
            o_sb = opool.tile([P, P], f32)
            nc.scalar.copy(o_sb[:], o_ps[:])
            pix0 = (g * 4 + t4) * P
            nc.sync.dma_start(out_ap[pix0 : pix0 + P, :], o_sb[:])


def build_nc(debug=False):
    nc = bacc.Bacc(
        "TRN2",
        target_bir_lowering=False,
        debug=False,
        enable_asserts=False,
        num_devices=8,
    )
    t_off = nc.dram_tensor("off", [NPIX, 2 * KN], f32, kind="ExternalInput")
    t_grid = nc.dram_tensor("grid", [P, NT * 2 * KN], f32, kind="ExternalInput")
    t_w = nc.dram_tensor("w", [C, KN, F], bf16, kind="ExternalInput")
    t_b = nc.dram_tensor("b", [F], f32, kind="ExternalInput")
    t_out = nc.dram_tensor("out", [NPIX, F], f32, kind="ExternalOutput")
    t_xp = nc.dram_tensor("xpair", [NPIX + 2, 2 * C], bf16, kind="ExternalInput")
    with tile.TileContext(nc) as tc:
        _body(tc, t_off, t_grid, t_w, t_b, t_out, t_xp, debug=debug)
    nc.compile()
    return nc


def make_in_maps(x, offset, W, b):
    B = x.shape[0]
    grid_host = _grid_const()
    import ml_dtypes
    w_host = np.ascontiguousarray(
        np.asarray(W, np.float32).transpose(1, 0, 2).astype(ml_dtypes.bfloat16))
    b_host = np.ascontiguousarray(np.asarray(b, np.float32))
    in_maps = []
    for i in range(B):
        xi = np.asarray(x[i], np.float32).reshape(NPIX, C).astype(ml_dtypes.bfloat16)
        xp = np.zeros((NPIX + 2, 2, C), ml_dtypes.bfloat16)
        xp[:NPIX, 0, :] = xi
        xp[: NPIX - 64, 1, :] = xi[64:]
        in_maps.append(
            {
                "xpair": np.ascontiguousarray(xp.reshape(NPIX + 2, 2 * C)),
                "off": np.ascontiguousarray(
                    np.asarray(offset[i], np.float32).reshape(NPIX, 2 * KN)
                ),
                "grid": grid_host,
                "w": w_host,
                "b": b_host,
            }
        )
    return in_maps


_RESULTS_CACHE = {}


def kernel(x, offset, W, b, _trace=False):
    x = np.asarray(x)
    B = x.shape[0]
    assert x.shape == (B, H, W_IMG, C), x.shape
    nc = build_nc()
    in_maps = make_in_maps(x, offset, W, b)
    res = run_bass_kernel_spmd(nc, in_maps, core_ids=list(range(B)), trace=_trace)
    _RESULTS_CACHE["last"] = res
    out = np.stack(
        [res.results[i]["out"].reshape(H, W_IMG, F) for i in range(B)]
    ).astype(np.float32)
    return out



# revision 19
# speedup vs baseline: 1.1294x; 1.1294x over previous
"""Trainium2 Bass kernel for DeformableConv2d (B,H,W,C=8,64,64,128; F=128; 3x3).

Strategy (data-parallel over batch, one batch element per NeuronCore):
  - Host: reshape inputs, precompute the (data-independent) base-grid constant
    and a channel-major weight layout. No data-dependent work on host.
  - Device, per core:
      * x_pair in DRAM: row q -> [x[q], x[q+64]] (zero padded), so one
        512-element contiguous read at offset q*256 fetches the whole 2x2
        bilinear patch for integer corner q = y0*64 + x0.
      * index math on DVE: coords = grid + offset, clip, frac via mod,
        q = y0*64 + x0 (int32), 4 bilinear corner weights (bf16).
      * per pixel-group (512 px = 4 tiles): ONE indirect DMA with 36
        offsets/partition (9 kernel points x 4 tiles) gathers [128, 36, 512]
        in a single Pool instruction (amortizes the ~1us SWDGE fixed cost
        that dominated the per-(tile,kn) gather variant).
      * corner combine on DVE with wide ops: one broadcast-weight multiply
        (w4 broadcast along channels via 0-stride AP) + pairwise add tree,
        split into two kn-halves so PE can start while DVE finishes.
      * PE transposes deform tiles into PSUM (channel-major); PE matmuls
        accumulate over the 9 kernel points into out^T; PE transposes back
        and bf16 results stream to DRAM.
"""

import os
from contextlib import ExitStack

import numpy as np

import concourse.bass as bass
import concourse.mybir as mybir
import concourse.tile as tile
from concourse import bacc, library_config
from concourse._compat import with_exitstack
from concourse.bass_utils import run_bass_kernel_spmd
from concourse.masks import make_identity

KH, KW, KN = 3, 3, 9
H = W_IMG = 64
C = 128
F = 128
P = 128
NPIX = H * W_IMG            # 4096 pixels per core
NT = NPIX // P              # 32 pixel tiles
NG = NT // 4                # 8 groups of 512 pixels
T4 = 4                      # tiles per group
NJ = KN * T4                # 36 gather indices per partition per group
KN_SPLIT = (slice(0, 5), slice(5, 9))  # kn halves for DVE/PE overlap

f32 = mybir.dt.float32
bf16 = mybir.dt.bfloat16
i32 = mybir.dt.int32
i16 = mybir.dt.int16
ALU = mybir.AluOpType
ACT = mybir.ActivationFunctionType


def _selmat_const():
    """Selection matrices S_a[p_in, p_out] = 1 iff p_in == 16a + (p_out % 16).
    matmul(lhsT=S_a, rhs=q) folds partition groups into the free dim to build
    the 16-partition-wrapped index layout dma_gather expects."""
    s = np.zeros((P, 8, P), np.float32)
    for a in range(8):
        for p_out in range(P):
            s[16 * a + (p_out % 16), a, p_out] = 1.0
    return s


def _grid_const():
    """reference._grid_offset in numpy, flattened to [4096, 18] then wrapped
    to the [128 partitions, 32*18] on-chip layout."""
    init = np.stack(np.meshgrid(np.arange(KH), np.arange(KW), indexing="ij"))
    init = init.reshape(-1, 2).astype(np.float32)
    ph, pw = (KH - 1) // 2, (KW - 1) // 2
    g = np.stack(
        np.meshgrid(np.arange(-ph, H - ph), np.arange(-pw, W_IMG - pw), indexing="ij"),
        axis=-1,
    ).astype(np.float32)
    full = (g[:, :, None, :] + init[None, None]).reshape(NPIX, 2 * KN)
    return np.ascontiguousarray(
        full.reshape(NT, P, 2 * KN).transpose(1, 0, 2).reshape(P, NT * 2 * KN)
    )


@with_exitstack
def _body(ctx: ExitStack, tc: "tile.TileContext", t_off, t_grid, t_w, t_b,
          t_out, t_xp, t_sel, debug=False):
    nc = tc.nc
    off_ap = t_off.ap()
    grid_ap = t_grid.ap()
    w_ap = t_w.ap()
    b_ap = t_b.ap()
    out_ap = t_out.ap()
    xp_ap = t_xp.ap()
    sel_ap = t_sel.ap()

    const = ctx.enter_context(tc.tile_pool(name="const", bufs=1))
    idxp = ctx.enter_context(tc.tile_pool(name="idx", bufs=1))
    gpool = ctx.enter_context(tc.tile_pool(name="gath", bufs=2))
    dpool = ctx.enter_context(tc.tile_pool(name="deform", bufs=2))
    dTpool = ctx.enter_context(tc.tile_pool(name="dT", bufs=3))
    oTpool = ctx.enter_context(tc.tile_pool(name="oT", bufs=2))
    opool = ctx.enter_context(tc.tile_pool(name="o", bufs=4))
    ps_out = ctx.enter_context(tc.tile_pool(name="ps_out", bufs=2, space="PSUM"))
    ps_dT = ctx.enter_context(tc.tile_pool(name="ps_dT", bufs=2, space="PSUM"))
    ps_o = ctx.enter_context(tc.tile_pool(name="ps_o", bufs=2, space="PSUM"))

    # ---- constants ----
    ident = const.tile([P, P], f32)
    make_identity(nc, ident[:])
    ident16 = const.tile([P, P], bf16)
    nc.vector.tensor_copy(ident16[:], ident[:])
    w_sb = const.tile([P, KN, F], bf16)
    nc.sync.dma_start(w_sb[:], w_ap)  # [C, KN, F] bf16, c on partitions
    b_sb = const.tile([P, 1], f32)
    nc.sync.dma_start(b_sb[:], b_ap[:, None])
    selmat = const.tile([P, 8, P], f32)
    nc.sync.dma_start(selmat[:], sel_ap.rearrange("p (a q) -> p a q", a=8))

    # x_pair viewed as overlapping 512-element rows at 256-element stride:
    # row q = [x[q], x[q+64], x[q+1], x[q+65]] = the 2x2 bilinear patch.
    xp_rows = t_xp.ap()
    xp_rows.ap = mybir.VecI64Pair([[256, NPIX + 1], [1, 512]])

    # ---- load offsets + grid ----
    offs = idxp.tile([P, NT, 2 * KN], f32)
    nc.sync.dma_start(offs[:], off_ap.rearrange("(t p) k -> p t k", p=P))
    grid = idxp.tile([P, NT, 2 * KN], f32)
    nc.sync.dma_start(grid[:], grid_ap.rearrange("p (t k) -> p t k", k=2 * KN))

    # ---- index math (all tiles at once) ----
    co = idxp.tile([P, NT, 2 * KN], f32)
    nc.vector.tensor_add(co[:], offs[:], grid[:])
    nc.vector.tensor_scalar(co[:], co[:], 0.0, float(H - 1), ALU.max, ALU.min)
    # floor via int round-trip; works for round-to-nearest (HW) and trunc (sim):
    # r = float(int(y)); floor = r - (r > y)
    ci = idxp.tile([P, NT, 2 * KN], i32)
    nc.vector.tensor_copy(ci[:], co[:])
    cf = idxp.tile([P, NT, 2 * KN], f32)
    nc.vector.tensor_copy(cf[:], ci[:])
    gt = idxp.tile([P, NT, 2 * KN], f32)
    nc.vector.tensor_tensor(gt[:], cf[:], co[:], ALU.is_gt)
    c0 = idxp.tile([P, NT, 2 * KN], f32)
    nc.vector.tensor_sub(c0[:], cf[:], gt[:])
    fr = idxp.tile([P, NT, 2 * KN], f32)
    nc.vector.tensor_sub(fr[:], co[:], c0[:])
    un = idxp.tile([P, NT, 2 * KN], f32)
    nc.vector.tensor_scalar(un[:], fr[:], -1.0, 1.0, ALU.mult, ALU.add)

    c0v = c0[:].rearrange("p t (n two) -> p t n two", two=2)

    qf = idxp.tile([P, NT, KN], f32)
    nc.vector.scalar_tensor_tensor(
        qf[:], c0v[:, :, :, 0], 64.0, c0v[:, :, :, 1], ALU.mult, ALU.add
    )
    # gather-order index tile as f32: [p, g, b] with b = kn*4 + t4
    qg = idxp.tile([P, NG, KN, T4], f32)
    nc.vector.tensor_copy(qg[:], qf[:].rearrange("p (g t) n -> p g n t", g=NG))
    # dma_gather wants idx n = b*128 + p at (partition n%16, slot n//16),
    # replicated across the 8 16-partition cores: T[16c + r, g, 8b + a] =
    # q[16a + r, g, b]. Build via selection matmuls that fold the partition
    # group a into the slot-minor position, then round f32 -> int16.
    qwrap = idxp.tile([P, NG, NJ, 8], i16)
    for a in range(8):
        q_ps = ps_o.tile([P, NG * NJ], f32)
        nc.tensor.matmul(q_ps[:], lhsT=selmat[:, a, :],
                         rhs=qg[:].rearrange("p g n t -> p (g n t)"),
                         start=True, stop=True)
        nc.vector.tensor_copy(
            qwrap[:, :, :, a],
            q_ps[:].rearrange("p (g j) -> p g j", g=NG))

    # corner weights [00, 10, 01, 11] in gather order [p, g, kn, t4, corner];
    # rows ~ y (index 0), cols ~ x (index 1)
    unr = un[:].rearrange("p (g t) (n two) -> p g n t two", g=NG, two=2)
    frr = fr[:].rearrange("p (g t) (n two) -> p g n t two", g=NG, two=2)
    w4 = idxp.tile([P, NG, KN, T4, 4], bf16)
    nc.vector.tensor_tensor(
        w4[:, :, :, :, 0], unr[:, :, :, :, 0], unr[:, :, :, :, 1], ALU.mult)
    nc.vector.tensor_tensor(
        w4[:, :, :, :, 1], frr[:, :, :, :, 0], unr[:, :, :, :, 1], ALU.mult)
    nc.vector.tensor_tensor(
        w4[:, :, :, :, 2], unr[:, :, :, :, 0], frr[:, :, :, :, 1], ALU.mult)
    nc.vector.tensor_tensor(
        w4[:, :, :, :, 3], frr[:, :, :, :, 0], frr[:, :, :, :, 1], ALU.mult)

    if debug:
        d_q = nc.dram_tensor("dbg_q", [P, NG * KN * T4], f32, kind="ExternalOutput")
        d_w4 = nc.dram_tensor("dbg_w4", [P, NG * KN * T4 * 4], bf16,
                              kind="ExternalOutput")
        d_g = nc.dram_tensor("dbg_g", [P, NJ * 512], bf16, kind="ExternalOutput")
        d_dt = nc.dram_tensor("dbg_dt", [P, KN * T4 * C], bf16, kind="ExternalOutput")
        nc.sync.dma_start(d_q.ap().rearrange("p (g n t) -> p g n t", g=NG, n=KN),
                          qg[:])
        nc.sync.dma_start(
            d_w4.ap().rearrange("p (g n t j) -> p g n t j", g=NG, n=KN, t=T4), w4[:])

    # ---- main loop ----
    for g in range(NG):
        # dma_gather in 2-block chunks (the SWDGE descriptor ring corrupts
        # beyond ~16 data descriptors per engine ring per instruction), cycled
        # over 4 SWDGE queues so descriptor generation runs on all 4 Q7 core
        # pairs in parallel. out[p, b, :] = xp_rows[idx[b*128 + p]]: 512
        # contiguous elements = rows q, q+1 of x_pair = the 2x2 patch.
        G = gpool.tile([P, NJ, 4, C], bf16)
        Gv512 = G[:].rearrange("p j k c -> p j (k c)")
        for j2 in range(NJ // 2):
            nc.gpsimd.dma_gather(
                out_ap=Gv512[:, 2 * j2:2 * j2 + 2],
                in_ap=xp_rows,
                idxs_ap=qwrap[:, g, 2 * j2:2 * j2 + 2].rearrange(
                    "p j a -> p (j a)"),
                num_idxs=2 * P,
                num_idxs_reg=2 * P,
                elem_size=512,
                elem_step=256,
                single_packet=False,
                queue_num=(g * (NJ // 2) + j2) % 4,
            )
        if debug and g == 0:
            nc.sync.dma_start(
                d_g.ap().rearrange("p (j k) -> p j k", j=NJ),
                G[:].rearrange("p j k c -> p j (k c)"))

        w4g = w4[:, g].rearrange("p n t j -> p (n t) j")  # [P, 36, 4]
        d_all = dpool.tile([P, KN, T4, C], bf16)
        ops = ps_out.tile([P, 512], f32)  # out^T accumulator [f, 512 px]
        for half, ksl in enumerate(KN_SPLIT):
            jsl = slice(ksl.start * T4, ksl.stop * T4)
            Gh = G[:, jsl]                         # [P, nj, 4, C]
            nj = jsl.stop - jsl.start
            w4b = w4g[:, jsl].to_broadcast([P, nj, 4, C])
            # weighted corners in place, then pairwise sum tree
            nc.vector.tensor_tensor(Gh, Gh, w4b, ALU.mult)
            nc.vector.tensor_add(Gh[:, :, 0, :], Gh[:, :, 0, :], Gh[:, :, 1, :])
            nc.vector.tensor_add(Gh[:, :, 2, :], Gh[:, :, 2, :], Gh[:, :, 3, :])
            d_h = d_all[:, ksl].rearrange("p n t c -> p (n t) c")
            nc.vector.tensor_add(d_h, Gh[:, :, 0, :], Gh[:, :, 2, :])
            for kn in range(ksl.start, ksl.stop):
                dps = ps_dT.tile([P, 512], bf16)  # deform^T [c, 512 px]
                for t4 in range(T4):
                    nc.tensor.transpose(
                        dps[:, t4 * P:(t4 + 1) * P], d_all[:, kn, t4, :],
                        ident16[:])
                dT = dTpool.tile([P, 512], bf16)
                nc.scalar.copy(dT[:], dps[:])
                nc.tensor.matmul(
                    ops[:], lhsT=w_sb[:, kn, :], rhs=dT[:],
                    start=(kn == 0), stop=(kn == KN - 1),
                )
        if debug and g == 0:
            nc.sync.dma_start(
                d_dt.ap().rearrange("p (n t c) -> p n t c", n=KN, t=T4), d_all[:])
        oT = oTpool.tile([P, 512], bf16)
        nc.scalar.activation(oT[:], ops[:], ACT.Identity, bias=b_sb[:, 0:1],
                             scale=1.0)
        for t4 in range(T4):
            o_ps = ps_o.tile([P, P], bf16)
            nc.tensor.transpose(o_ps[:], oT[:, t4 * P:(t4 + 1) * P], ident16[:])
            o_sb = opool.tile([P, P], bf16)
            nc.scalar.copy(o_sb[:], o_ps[:])
            pix0 = (g * 4 + t4) * P
            nc.sync.dma_start(out_ap[pix0:pix0 + P, :], o_sb[:])


def build_nc(debug=False):
    nc = bacc.Bacc(
        "TRN2",
        target_bir_lowering=False,
        debug=False,
        enable_asserts=False,
        num_devices=8,
        num_swdge_queues=4,
    )
    t_off = nc.dram_tensor("off", [NPIX, 2 * KN], f32, kind="ExternalInput")
    t_grid = nc.dram_tensor("grid", [P, NT * 2 * KN], f32, kind="ExternalInput")
    t_w = nc.dram_tensor("w", [C, KN, F], bf16, kind="ExternalInput")
    t_b = nc.dram_tensor("b", [F], f32, kind="ExternalInput")
    t_out = nc.dram_tensor("out", [NPIX, F], bf16, kind="ExternalOutput")
    t_xp = nc.dram_tensor("xpair", [NPIX + 2, 2 * C], bf16, kind="ExternalInput")
    t_sel = nc.dram_tensor("selmat", [P, 8 * P], f32, kind="ExternalInput")
    with tile.TileContext(nc) as tc:
        _body(tc, t_off, t_grid, t_w, t_b, t_out, t_xp, t_sel, debug=debug)
    nc.compile()
    return nc


def make_in_maps(x, offset, W, b):
    B = x.shape[0]
    grid_host = _grid_const()
    sel_host = np.ascontiguousarray(_selmat_const().reshape(P, 8 * P))
    import ml_dtypes
    w_host = np.ascontiguousarray(
        np.asarray(W, np.float32).transpose(1, 0, 2).astype(ml_dtypes.bfloat16))
    b_host = np.ascontiguousarray(np.asarray(b, np.float32))
    in_maps = []
    for i in range(B):
        xi = np.asarray(x[i], np.float32).reshape(NPIX, C).astype(ml_dtypes.bfloat16)
        xp = np.zeros((NPIX + 2, 2, C), ml_dtypes.bfloat16)
        xp[:NPIX, 0, :] = xi
        xp[: NPIX - 64, 1, :] = xi[64:]
        in_maps.append(
            {
                "xpair": np.ascontiguousarray(xp.reshape(NPIX + 2, 2 * C)),
                "off": np.ascontiguousarray(
                    np.asarray(offset[i], np.float32).reshape(NPIX, 2 * KN)
                ),
                "grid": grid_host,
                "w": w_host,
                "b": b_host,
                "selmat": sel_host,
            }
        )
    return in_maps


_RESULTS_CACHE = {}


def kernel(x, offset, W, b, _trace=False, _debug=False):
    x = np.asarray(x)
    B = x.shape[0]
    assert x.shape == (B, H, W_IMG, C), x.shape
    nc = build_nc(debug=_debug)
    in_maps = make_in_maps(x, offset, W, b)
    res = run_bass_kernel_spmd(nc, in_maps, core_ids=list(range(B)), trace=_trace)
    _RESULTS_CACHE["last"] = res
    out = np.stack(
        [res.results[i]["out"].reshape(H, W_IMG, F) for i in range(B)]
    ).astype(np.float32)
    return out


# revision 22
# speedup vs baseline: 1.4547x; 1.2880x over previous
"""Trainium2 Bass kernel for DeformableConv2d (B,H,W,C=8,64,64,128; F=128; 3x3).

Strategy (data-parallel over batch, one batch element per NeuronCore):
  - Host: reshape inputs, precompute the (data-independent) base-grid constant
    and a channel-major weight layout. No data-dependent work on host.
  - Device, per core:
      * x_pair in DRAM: row q -> [x[q], x[q+64]] (zero padded), so one
        512-element contiguous read at offset q*256 fetches the whole 2x2
        bilinear patch for integer corner q = y0*64 + x0.
      * index math on DVE: coords = grid + offset, clip, frac via mod,
        q = y0*64 + x0 (int32), 4 bilinear corner weights (bf16).
      * per pixel-group (512 px = 4 tiles): ONE indirect DMA with 36
        offsets/partition (9 kernel points x 4 tiles) gathers [128, 36, 512]
        in a single Pool instruction (amortizes the ~1us SWDGE fixed cost
        that dominated the per-(tile,kn) gather variant).
      * corner combine on DVE with wide ops: one broadcast-weight multiply
        (w4 broadcast along channels via 0-stride AP) + pairwise add tree,
        split into two kn-halves so PE can start while DVE finishes.
      * PE transposes deform tiles into PSUM (channel-major); PE matmuls
        accumulate over the 9 kernel points into out^T; PE transposes back
        and bf16 results stream to DRAM.
"""

import os
from contextlib import ExitStack

import numpy as np

import concourse.bass as bass
import concourse.mybir as mybir
import concourse.tile as tile
from concourse import bacc, library_config
from concourse._compat import with_exitstack
from concourse.bass_utils import run_bass_kernel_spmd
from concourse.masks import make_identity

KH, KW, KN = 3, 3, 9
H = W_IMG = 64
C = 128
F = 128
P = 128
NPIX = H * W_IMG            # 4096 pixels per core
NT = NPIX // P              # 32 pixel tiles
NG = NT // 4                # 8 groups of 512 pixels
T4 = 4                      # tiles per group
NJ = KN * T4                # 36 gather indices per partition per group
KN_SPLIT = (slice(0, 5), slice(5, 9))  # kn halves for DVE/PE overlap

f32 = mybir.dt.float32
bf16 = mybir.dt.bfloat16
i32 = mybir.dt.int32
i16 = mybir.dt.int16
ALU = mybir.AluOpType
ACT = mybir.ActivationFunctionType


def _selmat_const():
    """Selection matrices S_a[p_in, p_out] = 1 iff p_in == 16a + (p_out % 16).
    matmul(lhsT=S_a, rhs=q) folds partition groups into the free dim to build
    the 16-partition-wrapped index layout dma_gather expects."""
    s = np.zeros((P, 8, P), np.float32)
    for a in range(8):
        for p_out in range(P):
            s[16 * a + (p_out % 16), a, p_out] = 1.0
    return s


def _grid_const():
    """reference._grid_offset in numpy, flattened to [4096, 18] then wrapped
    to the [128 partitions, 32*18] on-chip layout."""
    init = np.stack(np.meshgrid(np.arange(KH), np.arange(KW), indexing="ij"))
    init = init.reshape(-1, 2).astype(np.float32)
    ph, pw = (KH - 1) // 2, (KW - 1) // 2
    g = np.stack(
        np.meshgrid(np.arange(-ph, H - ph), np.arange(-pw, W_IMG - pw), indexing="ij"),
        axis=-1,
    ).astype(np.float32)
    full = (g[:, :, None, :] + init[None, None]).reshape(NPIX, 2 * KN)
    return np.ascontiguousarray(
        full.reshape(NT, P, 2 * KN).transpose(1, 0, 2).reshape(P, NT * 2 * KN)
    )


@with_exitstack
def _body(ctx: ExitStack, tc: "tile.TileContext", t_off, t_grid, t_w, t_b,
          t_out, t_xp, t_sel, debug=False):
    nc = tc.nc
    off_ap = t_off.ap()
    grid_ap = t_grid.ap()
    w_ap = t_w.ap()
    b_ap = t_b.ap()
    out_ap = t_out.ap()
    xp_ap = t_xp.ap()
    sel_ap = t_sel.ap()

    const = ctx.enter_context(tc.tile_pool(name="const", bufs=1))
    idxp = ctx.enter_context(tc.tile_pool(name="idx", bufs=1))
    gpool = ctx.enter_context(tc.tile_pool(name="gath", bufs=2))
    dpool = ctx.enter_context(tc.tile_pool(name="deform", bufs=2))
    dTpool = ctx.enter_context(tc.tile_pool(name="dT", bufs=3))
    oTpool = ctx.enter_context(tc.tile_pool(name="oT", bufs=2))
    opool = ctx.enter_context(tc.tile_pool(name="o", bufs=4))
    ps_out = ctx.enter_context(tc.tile_pool(name="ps_out", bufs=2, space="PSUM"))
    ps_dT = ctx.enter_context(tc.tile_pool(name="ps_dT", bufs=2, space="PSUM"))
    ps_o = ctx.enter_context(tc.tile_pool(name="ps_o", bufs=2, space="PSUM"))

    # ---- constants ----
    ident = const.tile([P, P], f32)
    make_identity(nc, ident[:])
    ident16 = const.tile([P, P], bf16)
    nc.vector.tensor_copy(ident16[:], ident[:])
    w_sb = const.tile([P, KN, F], bf16)
    nc.sync.dma_start(w_sb[:], w_ap)  # [C, KN, F] bf16, c on partitions
    b_sb = const.tile([P, 1], f32)
    nc.sync.dma_start(b_sb[:], b_ap[:, None])
    selmat = const.tile([P, 8, P], f32)
    nc.sync.dma_start(selmat[:], sel_ap.rearrange("p (a q) -> p a q", a=8))

    # x_pair viewed as overlapping 512-element rows at 256-element stride:
    # row q = [x[q], x[q+64], x[q+1], x[q+65]] = the 2x2 bilinear patch.
    xp_rows = t_xp.ap()
    xp_rows.ap = mybir.VecI64Pair([[256, NPIX + 1], [1, 512]])

    # ---- load offsets + grid ----
    offs = idxp.tile([P, NT, 2 * KN], f32)
    nc.sync.dma_start(offs[:], off_ap.rearrange("(t p) k -> p t k", p=P))
    grid = idxp.tile([P, NT, 2 * KN], f32)
    nc.sync.dma_start(grid[:], grid_ap.rearrange("p (t k) -> p t k", k=2 * KN))

    # ---- index math (all tiles at once) ----
    co = idxp.tile([P, NT, 2 * KN], f32)
    nc.vector.tensor_add(co[:], offs[:], grid[:])
    nc.vector.tensor_scalar(co[:], co[:], 0.0, float(H - 1), ALU.max, ALU.min)
    # floor via int round-trip; works for round-to-nearest (HW) and trunc (sim):
    # r = float(int(y)); floor = r - (r > y)
    ci = idxp.tile([P, NT, 2 * KN], i32)
    nc.vector.tensor_copy(ci[:], co[:])
    cf = idxp.tile([P, NT, 2 * KN], f32)
    nc.vector.tensor_copy(cf[:], ci[:])
    gt = idxp.tile([P, NT, 2 * KN], f32)
    nc.vector.tensor_tensor(gt[:], cf[:], co[:], ALU.is_gt)
    c0 = idxp.tile([P, NT, 2 * KN], f32)
    nc.vector.tensor_sub(c0[:], cf[:], gt[:])
    fr = idxp.tile([P, NT, 2 * KN], f32)
    nc.vector.tensor_sub(fr[:], co[:], c0[:])
    un = idxp.tile([P, NT, 2 * KN], f32)
    nc.vector.tensor_scalar(un[:], fr[:], -1.0, 1.0, ALU.mult, ALU.add)

    c0v = c0[:].rearrange("p t (n two) -> p t n two", two=2)

    qf = idxp.tile([P, NT, KN], f32)
    nc.vector.scalar_tensor_tensor(
        qf[:], c0v[:, :, :, 0], 64.0, c0v[:, :, :, 1], ALU.mult, ALU.add
    )
    # gather-order index tile as f32: [p, g, b] with b = kn*4 + t4
    qg = idxp.tile([P, NG, KN, T4], f32)
    nc.vector.tensor_copy(qg[:], qf[:].rearrange("p (g t) n -> p g n t", g=NG))
    # dma_gather wants idx n = b*128 + p at (partition n%16, slot n//16),
    # replicated across the 8 16-partition cores: T[16c + r, g, 8b + a] =
    # q[16a + r, g, b]. Build via selection matmuls that fold the partition
    # group a into the slot-minor position, then round f32 -> int16.
    qwrap = idxp.tile([P, NG, NJ, 8], i16)
    for a in range(8):
        q_ps = ps_o.tile([P, NG * NJ], f32)
        nc.tensor.matmul(q_ps[:], lhsT=selmat[:, a, :],
                         rhs=qg[:].rearrange("p g n t -> p (g n t)"),
                         start=True, stop=True)
        nc.vector.tensor_copy(
            qwrap[:, :, :, a],
            q_ps[:].rearrange("p (g j) -> p g j", g=NG))

    # corner weights [00, 10, 01, 11] in gather order [p, g, kn, t4, corner];
    # rows ~ y (index 0), cols ~ x (index 1)
    unr = un[:].rearrange("p (g t) (n two) -> p g n t two", g=NG, two=2)
    frr = fr[:].rearrange("p (g t) (n two) -> p g n t two", g=NG, two=2)
    w4 = idxp.tile([P, NG, KN, T4, 4], bf16)
    nc.vector.tensor_tensor(
        w4[:, :, :, :, 0], unr[:, :, :, :, 0], unr[:, :, :, :, 1], ALU.mult)
    nc.vector.tensor_tensor(
        w4[:, :, :, :, 1], frr[:, :, :, :, 0], unr[:, :, :, :, 1], ALU.mult)
    nc.vector.tensor_tensor(
        w4[:, :, :, :, 2], unr[:, :, :, :, 0], frr[:, :, :, :, 1], ALU.mult)
    nc.vector.tensor_tensor(
        w4[:, :, :, :, 3], frr[:, :, :, :, 0], frr[:, :, :, :, 1], ALU.mult)
    # 8-wide replication so the weighted-corner multiply reads a real
    # contiguous 8-run (full 0-stride broadcast measured 2.2x slower on DVE)
    w4x8 = idxp.tile([P, NG, KN, T4, 4, 8], bf16)
    nc.vector.tensor_copy(
        w4x8[:].rearrange("p g n t k e -> p (g n t k) e"),
        w4[:].rearrange("p g n t k -> p (g n t k)").to_broadcast(
            [P, NG * KN * T4 * 4, 8]))

    if debug:
        d_q = nc.dram_tensor("dbg_q", [P, NG * KN * T4], f32, kind="ExternalOutput")
        d_w4 = nc.dram_tensor("dbg_w4", [P, NG * KN * T4 * 4], bf16,
                              kind="ExternalOutput")
        d_g = nc.dram_tensor("dbg_g", [P, NJ * 512], bf16, kind="ExternalOutput")
        d_dt = nc.dram_tensor("dbg_dt", [P, KN * T4 * C], bf16, kind="ExternalOutput")
        nc.sync.dma_start(d_q.ap().rearrange("p (g n t) -> p g n t", g=NG, n=KN),
                          qg[:])
        nc.sync.dma_start(
            d_w4.ap().rearrange("p (g n t j) -> p g n t j", g=NG, n=KN, t=T4), w4[:])

    # ---- main loop ----
    for g in range(NG):
        # dma_gather in 2-block chunks (the SWDGE descriptor ring corrupts
        # beyond ~16 data descriptors per engine ring per instruction), cycled
        # over 4 SWDGE queues so descriptor generation runs on all 4 Q7 core
        # pairs in parallel. out[p, b, :] = xp_rows[idx[b*128 + p]]: 512
        # contiguous elements = rows q, q+1 of x_pair = the 2x2 patch.
        G = gpool.tile([P, NJ, 4, C], bf16)
        Gv512 = G[:].rearrange("p j k c -> p j (k c)")
        for j2 in range(NJ // 2):
            nc.gpsimd.dma_gather(
                out_ap=Gv512[:, 2 * j2:2 * j2 + 2],
                in_ap=xp_rows,
                idxs_ap=qwrap[:, g, 2 * j2:2 * j2 + 2].rearrange(
                    "p j a -> p (j a)"),
                num_idxs=2 * P,
                num_idxs_reg=2 * P,
                elem_size=512,
                elem_step=256,
                single_packet=False,
                queue_num=(g * (NJ // 2) + j2) % 4,
            )
        if debug and g == 0:
            nc.sync.dma_start(
                d_g.ap().rearrange("p (j k) -> p j k", j=NJ),
                G[:].rearrange("p j k c -> p j (k c)"))

        d_all = dpool.tile([P, KN, T4, C], bf16)
        ops = ps_out.tile([P, 512], f32)  # out^T accumulator [f, 512 px]
        for half, ksl in enumerate(KN_SPLIT):
            jsl = slice(ksl.start * T4, ksl.stop * T4)
            Gh = G[:, jsl]                         # [P, nj, 4, C]
            nj = jsl.stop - jsl.start
            # weight operand: real 8-run + 0-stride 16 in the middle
            w4v = w4x8[:, g, ksl].rearrange("p n t k e -> p (n t k) e")
            apl = [list(x) for x in w4v.ap]
            w4v.ap = mybir.VecI64Pair(
                [apl[0], apl[1], [0, 16], apl[2]])
            Ghv = Gh.rearrange("p j k (s e) -> p (j k) s e", s=16)
            # weighted corners in place, then pairwise sum tree
            nc.vector.tensor_tensor(Ghv, Ghv, w4v, ALU.mult)
            nc.vector.tensor_add(Gh[:, :, 0, :], Gh[:, :, 0, :], Gh[:, :, 1, :])
            nc.vector.tensor_add(Gh[:, :, 2, :], Gh[:, :, 2, :], Gh[:, :, 3, :])
            d_h = d_all[:, ksl].rearrange("p n t c -> p (n t) c")
            nc.vector.tensor_add(d_h, Gh[:, :, 0, :], Gh[:, :, 2, :])
            for kn in range(ksl.start, ksl.stop):
                dps = ps_dT.tile([P, 512], bf16)  # deform^T [c, 512 px]
                for t4 in range(T4):
                    nc.tensor.transpose(
                        dps[:, t4 * P:(t4 + 1) * P], d_all[:, kn, t4, :],
                        ident16[:])
                dT = dTpool.tile([P, 512], bf16)
                nc.scalar.copy(dT[:], dps[:])
                nc.tensor.matmul(
                    ops[:], lhsT=w_sb[:, kn, :], rhs=dT[:],
                    start=(kn == 0), stop=(kn == KN - 1),
                )
        if debug and g == 0:
            nc.sync.dma_start(
                d_dt.ap().rearrange("p (n t c) -> p n t c", n=KN, t=T4), d_all[:])
        oT = oTpool.tile([P, 512], bf16)
        nc.scalar.activation(oT[:], ops[:], ACT.Identity, bias=b_sb[:, 0:1],
                             scale=1.0)
        for t4 in range(T4):
            o_ps = ps_o.tile([P, P], bf16)
            nc.tensor.transpose(o_ps[:], oT[:, t4 * P:(t4 + 1) * P], ident16[:])
            o_sb = opool.tile([P, P], bf16)
            nc.scalar.copy(o_sb[:], o_ps[:])
            pix0 = (g * 4 + t4) * P
            nc.sync.dma_start(out_ap[pix0:pix0 + P, :], o_sb[:])


def build_nc(debug=False):
    nc = bacc.Bacc(
        "TRN2",
        target_bir_lowering=False,
        debug=False,
        enable_asserts=False,
        num_devices=8,
        num_swdge_queues=4,
    )
    t_off = nc.dram_tensor("off", [NPIX, 2 * KN], f32, kind="ExternalInput")
    t_grid = nc.dram_tensor("grid", [P, NT * 2 * KN], f32, kind="ExternalInput")
    t_w = nc.dram_tensor("w", [C, KN, F], bf16, kind="ExternalInput")
    t_b = nc.dram_tensor("b", [F], f32, kind="ExternalInput")
    t_out = nc.dram_tensor("out", [NPIX, F], bf16, kind="ExternalOutput")
    t_xp = nc.dram_tensor("xpair", [NPIX + 2, 2 * C], bf16, kind="ExternalInput")
    t_sel = nc.dram_tensor("selmat", [P, 8 * P], f32, kind="ExternalInput")
    with tile.TileContext(nc) as tc:
        _body(tc, t_off, t_grid, t_w, t_b, t_out, t_xp, t_sel, debug=debug)
    nc.compile()
    return nc


def make_in_maps(x, offset, W, b):
    B = x.shape[0]
    grid_host = _grid_const()
    sel_host = np.ascontiguousarray(_selmat_const().reshape(P, 8 * P))
    import ml_dtypes
    w_host = np.ascontiguousarray(
        np.asarray(W, np.float32).transpose(1, 0, 2).astype(ml_dtypes.bfloat16))
    b_host = np.ascontiguousarray(np.asarray(b, np.float32))
    in_maps = []
    for i in range(B):
        xi = np.asarray(x[i], np.float32).reshape(NPIX, C).astype(ml_dtypes.bfloat16)
        xp = np.zeros((NPIX + 2, 2, C), ml_dtypes.bfloat16)
        xp[:NPIX, 0, :] = xi
        xp[: NPIX - 64, 1, :] = xi[64:]
        in_maps.append(
            {
                "xpair": np.ascontiguousarray(xp.reshape(NPIX + 2, 2 * C)),
                "off": np.ascontiguousarray(
                    np.asarray(offset[i], np.float32).reshape(NPIX, 2 * KN)
                ),
                "grid": grid_host,
                "w": w_host,
                "b": b_host,
                "selmat": sel_host,
            }
        )
    return in_maps


_RESULTS_CACHE = {}


def kernel(x, offset, W, b, _trace=False, _debug=False):
    x = np.asarray(x)
    B = x.shape[0]
    assert x.shape == (B, H, W_IMG, C), x.shape
    nc = build_nc(debug=_debug)
    in_maps = make_in_maps(x, offset, W, b)
    res = run_bass_kernel_spmd(nc, in_maps, core_ids=list(range(B)), trace=_trace)
    _RESULTS_CACHE["last"] = res
    out = np.stack(
        [res.results[i]["out"].reshape(H, W_IMG, F) for i in range(B)]
    ).astype(np.float32)
    return out
